# revision 1
# baseline (speedup 1.0000x reference)
"""Trainium2 Bass kernel for the 2-player masked LSTM scan.

Reference semantics (T=128 steps, B=256 batch, C=1024 in, H=1024 hidden):
  per step t, batch b: the active player's (c,h) (selected by main[t,b]) runs
  one LSTM cell z = x@Wi + h@Wh + b with fused i,f,g,o gates; the result is
  written back only to the active player's state, and both players' states are
  zeroed where done[t,b].

Key algorithmic idea: done/main are *inputs*, so the true dependency structure
is known on the host.  Each (b, segment, player) triple forms an independent
"chain" of positions; a position at depth d in its chain depends only on the
position at depth d-1.  With done ~ Bernoulli(0.5) per step, chains are short
(max depth ~17 for the target inputs), so the sequential scan of 128 steps
becomes ~17 dense "waves", each a full-batch matmul with no masking at all.

Host side: sort chains by length (desc), round-robin across the 8 cores, lay
positions out wave-major.  Because chain order is sorted by length, the chains
alive at depth d are exactly a prefix of those alive at depth d-1 - so wave d
reads a contiguous prefix of wave d-1's outputs: no gather needed on device.

Device: phase A computes zx = x@Wi (+ bias via DVE) for all positions, fusing
the full gate math for depth-0 positions (input state is zero); phase B runs
one matmul z = zx + h@Wh per wave plus the LSTM gate math.  bf16 matmuls with
fp32 PSUM accumulation; the carried cell state c stays fp32.  Single-m-tile
waves keep all state in SBUF (PE transpose for h^T) so the PE never idles
long enough for the HAM clock gate to re-throttle.
"""

import sys

sys.path.insert(0, "/opt/trn_rl_repo")

import numpy as np
import ml_dtypes

import concourse.bass as bass
import concourse.tile as tile
from concourse import bacc, mybir
from concourse.bass_utils import run_bass_kernel_spmd

BF16 = ml_dtypes.bfloat16
AF = mybir.ActivationFunctionType
DT = mybir.dt

NCORES = 8
H = 1024
CIN = 1024
G = 4 * H  # 4096 fused gate width
KT = CIN // 128  # 8 k-tiles for both Wi and Wh contractions


# ---------------------------------------------------------------------------
# Host-side schedule construction
# ---------------------------------------------------------------------------

def _build_schedule(done, main, T, B):
    """Chain decomposition of the (t, b) grid.

    Returns per-position (core, depth, rank) and the uniform padded wave
    geometry shared by all cores (SPMD requires identical programs).
    """
    done2 = done.reshape(T, B).astype(bool)
    main2 = main.reshape(T, B).astype(bool)

    seg = np.zeros((T, B), np.int64)
    if T > 1:
        seg[1:] = np.cumsum(done2[:-1], axis=0)
    player = main2.astype(np.int64)
    key = (np.arange(B)[None, :] * (T + 1) + seg) * 2 + player  # [T, B]
    flat_key = key.reshape(-1)  # position p = t*B + b
    order = np.argsort(flat_key, kind="stable")  # chain-major, t-ascending
    sorted_keys = flat_key[order]
    uk, first_idx, inv = np.unique(sorted_keys, return_index=True, return_inverse=True)
    chain_len = np.diff(np.append(first_idx, len(sorted_keys)))
    npos = T * B

    depth = np.empty(npos, np.int64)
    depth[order] = np.arange(npos) - first_idx[inv]
    chain_id = np.empty(npos, np.int64)
    chain_id[order] = inv

    n_chains = len(uk)
    chain_b = uk // (2 * (T + 1))
    chain_seg = (uk // 2) % (T + 1)
    chain_player = uk % 2

    chain_order = np.argsort(-chain_len, kind="stable")
    rank_of_chain = np.empty(n_chains, np.int64)
    rank_of_chain[chain_order] = np.arange(n_chains)
    core_of_chain = (rank_of_chain % NCORES).astype(np.int64)
    core_rank = rank_of_chain // NCORES

    D = int(chain_len.max())
    lens_sorted = np.sort(chain_len)
    N_d = np.array([n_chains - np.searchsorted(lens_sorted, d, side="right")
                    for d in range(D)], np.int64)
    U = np.ceil(N_d / NCORES).astype(np.int64)      # uniform per-core wave rows
    M = np.ceil(U / 128).astype(np.int64)           # padded wave m-tiles
    V = np.concatenate([[0], np.cumsum(U)])          # packed row offsets
    P = np.concatenate([[0], np.cumsum(M * 128)])    # padded row offsets

    return dict(
        depth=depth, chain_id=chain_id, core_of_chain=core_of_chain,
        core_rank=core_rank, chain_b=chain_b, chain_seg=chain_seg,
        chain_player=chain_player, D=D, U=U, M=M, V=V, P=P,
    )


def _prep_inputs(x, c1, h1, c2, h2, Wi, Wh, b, done, main):
    """Build per-core device input arrays + output scatter indices."""
    B = c1.shape[0]
    T = x.shape[0] // B
    sch = _build_schedule(np.asarray(done), np.asarray(main), T, B)
    D, U, M, V, P = sch["D"], sch["U"], sch["M"], sch["V"], sch["P"]

    zero_init = not (np.any(c1) or np.any(h1) or np.any(c2) or np.any(h2))

    packed_total = int(V[D])
    need = packed_total
    for d in range(1 if zero_init else 0, D):
        need = max(need, int(V[d]) + int(M[d]) * 128)
    Mzx = (need + 127) // 128
    zx_row0 = int(V[1]) // 128 * 128 if (zero_init and D > 1) else 0
    zx_start_tile = zx_row0 // 128

    depth = sch["depth"]; chain_id = sch["chain_id"]
    core_pos = sch["core_of_chain"][chain_id]
    packed_row = V[depth] + sch["core_rank"][chain_id]
    padded_row = P[depth] + sch["core_rank"][chain_id]

    x = np.ascontiguousarray(np.asarray(x, np.float32))
    xt_blocks = []
    for c in range(NCORES):
        sel = core_pos == c
        Xp = np.zeros((Mzx * 128, CIN), np.float32)
        Xp[packed_row[sel]] = x[sel]
        # lhsT block layout: [mt, p, k*128 + m] = Xp[mt*128+m, k*128+p]
        xt = Xp.reshape(Mzx, 128, KT, 128).transpose(0, 3, 2, 1).reshape(Mzx, 128, CIN)
        xt_blocks.append(np.ascontiguousarray(xt.astype(BF16)))

    # weight layout per k-slice: [k, p, n] = W[k*128+p, n]
    Wi_l = np.ascontiguousarray(
        np.asarray(Wi, np.float32).reshape(KT, 128, G).astype(BF16))
    Wh_l = np.ascontiguousarray(
        np.asarray(Wh, np.float32).reshape(KT, 128, G).astype(BF16))
    bbc = np.ascontiguousarray(
        np.broadcast_to(np.asarray(b, np.float32)[None, :], (128, G)).astype(BF16))
    ident = np.ascontiguousarray(np.eye(128, dtype=np.float32).astype(BF16))

    ht0_blocks = [None] * NCORES
    c0_blocks = [None] * NCORES
    if not zero_init:
        h1 = np.asarray(h1, np.float32); h2 = np.asarray(h2, np.float32)
        c1 = np.asarray(c1, np.float32); c2 = np.asarray(c2, np.float32)
        hin = np.where(sch["chain_player"][:, None] > 0, h1[sch["chain_b"]],
                       h2[sch["chain_b"]])
        cin_ = np.where(sch["chain_player"][:, None] > 0, c1[sch["chain_b"]],
                        c2[sch["chain_b"]])
        live = sch["chain_seg"] == 0
        hin = np.where(live[:, None], hin, 0.0)
        cin_ = np.where(live[:, None], cin_, 0.0)
        M0 = int(M[0])
        for c in range(NCORES):
            selc = sch["core_of_chain"] == c
            rows = sch["core_rank"][selc]
            Hp = np.zeros((M0 * 128, H), np.float32)
            Cp = np.zeros((M0 * 128, H), np.float32)
            Hp[rows] = hin[selc]
            Cp[rows] = cin_[selc]
            # transposed layout for lhsT: [k, p, col] = Hp[col, k*128+p]
            ht0 = Hp.reshape(M0 * 128, KT, 128).transpose(1, 2, 0).astype(BF16)
            ht0_blocks[c] = np.ascontiguousarray(ht0)
            c0_blocks[c] = np.ascontiguousarray(Cp)

    return dict(
        sch=sch, zero_init=zero_init, Mzx=Mzx, zx_row0=zx_row0,
        zx_start_tile=zx_start_tile, xt_blocks=xt_blocks, Wi_l=Wi_l, Wh_l=Wh_l,
        bbc=bbc, ident=ident, ht0_blocks=ht0_blocks, c0_blocks=c0_blocks,
        core_pos=core_pos, padded_row=padded_row, T=T, B=B,
    )


# ---------------------------------------------------------------------------
# Device program
# ---------------------------------------------------------------------------

def _build_program(D, U, M, V, P, Mzx, zx_row0, zx_start_tile, zero_init,
                   no_tail=False):
    nc = bacc.Bacc("TRN2", target_bir_lowering=False, debug=False)

    M0 = int(M[0])
    Ptot = int(P[D])
    d_start = 1 if zero_init else 0
    need_zx = D > d_start
    nzx_rows = Mzx * 128 - zx_row0

    xt_d = nc.dram_tensor("xt", [Mzx, 128, CIN], DT.bfloat16, kind="ExternalInput")
    wi_d = nc.dram_tensor("wi", [KT, 128, G], DT.bfloat16, kind="ExternalInput")
    wh_d = nc.dram_tensor("wh", [KT, 128, G], DT.bfloat16, kind="ExternalInput")
    bbc_d = nc.dram_tensor("bbc", [128, G], DT.bfloat16, kind="ExternalInput")
    id_d = nc.dram_tensor("ident", [128, 128], DT.bfloat16, kind="ExternalInput")
    y_d = nc.dram_tensor("y", [Ptot, H], DT.float32, kind="ExternalOutput")
    if need_zx:
        zx_d = nc.dram_tensor("zx", [max(nzx_rows, 128), G], DT.bfloat16,
                              kind="Internal")
        zx_ap = zx_d.ap()
    hs_d = nc.dram_tensor("hstate", [Ptot, H], DT.bfloat16, kind="Internal")
    cs_d = nc.dram_tensor("cstate", [Ptot, H], DT.float32, kind="Internal")
    if not zero_init:
        ht0_d = nc.dram_tensor("ht0", [KT, 128, M0 * 128], DT.bfloat16,
                               kind="ExternalInput")
        c0_d = nc.dram_tensor("c0", [M0 * 128, H], DT.float32, kind="ExternalInput")

    xt_ap = xt_d.ap(); y_ap = y_d.ap(); hs_ap = hs_d.ap(); cs_ap = cs_d.ap()

    # Waves that keep state SBUF-resident: single m-tile, and the previous
    # wave small enough that its first m-tile's state tiles are still alive
    # (work-pool bufs=3 below).  M is monotone nonincreasing, so once a wave
    # qualifies every later wave does too.
    sbuf_in = [False] * D   # wave d takes h/c from prev wave's SBUF tiles
    for d in range(d_start + 1, D):
        if int(M[d]) == 1 and int(M[d - 1]) <= 3:
            sbuf_in[d] = True
    import os
    if no_tail or os.environ.get("KERNEL_NO_SBUF_TAIL"):
        sbuf_in = [False] * D

    with tile.TileContext(nc) as tc:
        from contextlib import ExitStack
        with ExitStack() as es:
            const = es.enter_context(tc.tile_pool(name="const", bufs=1))
            work = es.enter_context(tc.tile_pool(name="work", bufs=2))
            psum = es.enter_context(tc.tile_pool(name="psum", bufs=1, space="PSUM"))
            whPa = es.enter_context(tc.tile_pool(name="whPa", bufs=1))

            ident_sb = const.tile([128, 128], DT.bfloat16, tag="ident")
            # Wh k-slices 0..3 prefetched alongside Wi (fits in SBUF); 4..7
            # loaded once Wi's pool is released, overlapping the first waves.
            # (DMAs for these are emitted a few m-tiles into phase A so the
            # first x/Wi tiles win the DMA queues and the PE starts early.)
            wh_a = whPa.tile([128, 4 * G], DT.bfloat16, tag="wha", name="wh_a")

            def gate_math(z_src, c_src, out_row, store_h, store_c):
                """LSTM gate math for one 128-row m-tile.

                z_src: [128, G] bf16 SBUF tile holding z
                c_src: [128, H] fp32 ap with previous c, or None (c == 0)
                Returns (ncv, nhb) tiles (fp32 cell state, bf16 hidden).
                """
                gi = work.tile([128, H], DT.bfloat16, tag="gi", name="gi", bufs=1)
                gf = work.tile([128, H], DT.bfloat16, tag="gf", name="gf", bufs=1)
                gg = work.tile([128, H], DT.bfloat16, tag="gg", name="gg", bufs=1)
                go = work.tile([128, H], DT.bfloat16, tag="go", name="go", bufs=1)
                nc.scalar.activation(gi[:], z_src[:, 0 * H:1 * H], AF.Sigmoid)
                nc.scalar.activation(gf[:], z_src[:, 1 * H:2 * H], AF.Sigmoid)
                nc.scalar.activation(gg[:], z_src[:, 2 * H:3 * H], AF.Tanh)
                nc.scalar.activation(go[:], z_src[:, 3 * H:4 * H], AF.Sigmoid)

                if c_src is not None:
                    m1 = work.tile([128, H], DT.float32, tag="f32a", name="m1")
                    nc.vector.tensor_mul(m1[:], gi[:], gg[:])
                    t1 = work.tile([128, H], DT.float32, tag="f32b", name="t1")
                    nc.vector.tensor_mul(t1[:], gf[:], c_src[:])
                    ncv = work.tile([128, H], DT.float32, tag="ncv", name="ncv",
                                    bufs=3)
                    nc.vector.tensor_add(ncv[:], t1[:], m1[:])
                else:
                    ncv = work.tile([128, H], DT.float32, tag="ncv", name="ncv",
                                    bufs=3)
                    nc.vector.tensor_mul(ncv[:], gi[:], gg[:])
                tnc = work.tile([128, H], DT.float32, tag="f32b", name="tnc")
                nc.scalar.activation(tnc[:], ncv[:], AF.Tanh)
                nh = work.tile([128, H], DT.float32, tag="f32a", name="nh")
                nc.vector.tensor_mul(nh[:], go[:], tnc[:])
                nhb = work.tile([128, H], DT.bfloat16, tag="nhb", name="nhb",
                                bufs=3)
                nc.vector.tensor_copy(nhb[:], nh[:])

                nc.gpsimd.dma_start(out=y_ap[out_row:out_row + 128, :], in_=nh[:])
                if store_h:
                    nc.gpsimd.dma_start(out=hs_ap[out_row:out_row + 128, :],
                                        in_=nhb[:])
                if store_c:
                    nc.gpsimd.dma_start(out=cs_ap[out_row:out_row + 128, :],
                                        in_=ncv[:])
                return ncv, nhb

            def mm_half(lhsT_of_k, rhs_of_k, half, inject=None, k_order=None):
                """Half an m-tile of z accumulation (512 of 1024 cols/gate).

                Half-width PSUM tiles with bufs=2 let consecutive halves /
                m-tiles / waves double-buffer: the next half's matmuls run
                while this half's PSUM is still being drained by DVE/ACT.
                If inject is given (bf16 [128, G] SBUF tile), its half-slices
                are added into PSUM via identity matmuls (z += zx).
                """
                pt = []
                for g in range(4):
                    pt.append(psum.tile([128, 512], DT.float32, tag=f"ph{g}",
                                        name=f"ph{g}", bufs=2))
                ks = list(k_order) if k_order is not None else list(range(KT))
                for i, k in enumerate(ks):
                    lhsT = lhsT_of_k(k)
                    for g in range(4):
                        col0 = g * H + half * 512
                        nc.tensor.matmul(
                            pt[g][:], lhsT=lhsT, rhs=rhs_of_k(k, col0),
                            start=(i == 0),
                            stop=(inject is None and i == len(ks) - 1),
                            skip_group_check=True)
                if inject is not None:
                    for g in range(4):
                        col0 = g * H + half * 512
                        nc.tensor.matmul(
                            pt[g][:], lhsT=ident_sb[:],
                            rhs=inject[:, col0:col0 + 512],
                            start=False, stop=True, skip_group_check=True)
                return pt

            # ---------------- phase A: zx = x@Wi + bias ----------------
            with tc.tile_pool(name="wiP", bufs=1) as wiP, \
                 tc.tile_pool(name="xtP", bufs=3) as xtP:
                wi_sb = wiP.tile([128, KT * G], DT.bfloat16, tag="wi", name="wi_sb")
                # emission order = DMA priority: k=0 and the first x tile win
                # the queues so the PE can start ~5us in
                nc.gpsimd.dma_start(out=wi_sb[:, 0:G], in_=wi_d.ap()[0])
                xt0_sb = xtP.tile([128, CIN], DT.bfloat16, tag="xt", name="xt_sb")
                nc.gpsimd.dma_start(out=xt0_sb[:], in_=xt_ap[0])
                for k in range(1, KT):
                    nc.gpsimd.dma_start(out=wi_sb[:, k * G:(k + 1) * G],
                                        in_=wi_d.ap()[k])
                bbc_sb = wiP.tile([128, G], DT.bfloat16, tag="bbc", name="bbc_sb")
                nc.gpsimd.dma_start(out=bbc_sb[:], in_=bbc_d.ap()[:])
                nc.gpsimd.dma_start(out=ident_sb[:], in_=id_d.ap()[:])

                for mt in range(Mzx):
                    if mt == min(3, Mzx - 1) and (need_zx or D > 1):
                        # prefetch Wh k=4..7 into wh_a (see WH_ORDER below)
                        for k in range(4):
                            nc.gpsimd.dma_start(out=wh_a[:, k * G:(k + 1) * G],
                                                in_=wh_d.ap()[4 + k])
                    if mt == 0:
                        xt_sb = xt0_sb
                    else:
                        xt_sb = xtP.tile([128, CIN], DT.bfloat16, tag="xt",
                                         name="xt_sb")
                        nc.gpsimd.dma_start(out=xt_sb[:], in_=xt_ap[mt])

                    zxt = work.tile([128, G], DT.bfloat16, tag="zws", name="zxt")
                    for half in range(2):
                        pt = mm_half(
                            lambda k: xt_sb[:, k * 128:(k + 1) * 128],
                            lambda k, col0: wi_sb[:, k * G + col0:
                                                  k * G + col0 + 512],
                            half)
                        # psum -> SBUF with bias add (bf16)
                        for g in range(4):
                            col0 = g * H + half * 512
                            nc.vector.tensor_add(zxt[:, col0:col0 + 512],
                                                 pt[g][:],
                                                 bbc_sb[:, col0:col0 + 512])

                    if need_zx and mt >= zx_start_tile:
                        r = mt * 128 - zx_row0
                        nc.gpsimd.dma_start(out=zx_ap[r:r + 128, :], in_=zxt[:])
                    if zero_init and mt < M0:
                        nwave1 = int(M[1]) if D > 1 else 0
                        gate_math(zxt, None, mt * 128,
                                  store_h=(D > 1 and not sbuf_in[1]
                                           and mt < nwave1),
                                  store_c=(D > 1 and not sbuf_in[1]
                                           and mt < nwave1))

            # ---------------- phase B: waves ----------------
            if D > d_start:
                with tc.tile_pool(name="whPb", bufs=1) as whPb, \
                     tc.tile_pool(name="hTP", bufs=2) as hTP, \
                     tc.tile_pool(name="zxP", bufs=2) as zxP, \
                     tc.tile_pool(name="cP", bufs=2) as cP:
                    wh_b = whPb.tile([128, 4 * G], DT.bfloat16, tag="whb",
                                     name="wh_b")
                    for k in range(4):
                        nc.gpsimd.dma_start(out=wh_b[:, k * G:(k + 1) * G],
                                            in_=wh_d.ap()[k])

                    # wh_a (prefetched in phase A) holds k=4..7; wh_b loads
                    # k=0..3 at the phase boundary.  Waves iterate k starting
                    # at 4 so the first matmuls never wait on wh_b's DMA.
                    WH_ORDER = [4, 5, 6, 7, 0, 1, 2, 3]

                    def wh_rhs(k, col0):
                        if k >= 4:
                            return wh_a[:, (k - 4) * G + col0:
                                        (k - 4) * G + col0 + 512]
                        return wh_b[:, k * G + col0:k * G + col0 + 512]

                    hT_cols_max = KT * int(M[max(d_start, 1)]) * 128 \
                        if D > max(d_start, 1) else KT * 128
                    prev_ncv = None
                    prev_nhb = None
                    pending_hT = None  # hT pre-built by a tail wave for d+1

                    for d in range(d_start, D):
                        Md = int(M[d])
                        ncols = Md * 128

                        if sbuf_in[d]:
                            # ---- SBUF-resident tail wave (one m-tile) ----
                            zx_sb = zxP.tile([128, G], DT.bfloat16, tag="zx",
                                             name="zx_sb")
                            r = int(V[d]) - zx_row0
                            nc.gpsimd.dma_start(out=zx_sb[:],
                                                in_=zx_ap[r:r + 128, :])

                            import os as _os2
                            _no_xbar = bool(
                                _os2.environ.get("KERNEL_TAIL_NO_XBAR"))
                            if pending_hT is not None:
                                hT = pending_hT
                            else:
                                hT = hTP.tile([128, KT * 128], DT.bfloat16,
                                              tag="hTs", name="hTs")
                                if _no_xbar:
                                    for kk in range(KT):
                                        bk = slice(kk * 128, (kk + 1) * 128)
                                        nc.sync.dma_start(
                                            out=hT[:, bk],
                                            in_=prev_nhb[:, bk].rearrange(
                                                "a b -> b a"))
                                else:
                                    nc.sync.dma_start_transpose(
                                        out=hT.rearrange("p (j c) -> p j c",
                                                         c=128),
                                        in_=prev_nhb[:])

                            gi = work.tile([128, H], DT.bfloat16, tag="gi",
                                           name="gi", bufs=1)
                            gf = work.tile([128, H], DT.bfloat16, tag="gf",
                                           name="gf", bufs=1)
                            gg = work.tile([128, H], DT.bfloat16, tag="gg",
                                           name="gg", bufs=1)
                            go = work.tile([128, H], DT.bfloat16, tag="go",
                                           name="go", bufs=1)
                            ncv = work.tile([128, H], DT.float32, tag="ncv",
                                            name="ncv", bufs=3)
                            nh = work.tile([128, H], DT.float32, tag="f32a",
                                           name="nh")
                            nhb = work.tile([128, H], DT.bfloat16, tag="nhb",
                                            name="nhb", bufs=3)
                            build_next = d + 1 < D
                            if build_next:
                                hT_next = hTP.tile([128, KT * 128], DT.bfloat16,
                                                   tag="hTs", name="hTn")
                            # half-pass z accumulation + 256-col-block gate
                            # chains: each block's h^T transpose is issued as
                            # soon as its nhb quarter is ready, so the next
                            # wave's matmuls overlap this wave's gate math
                            lhsT_of_k = (lambda hh: lambda k:
                                         hh[:, k * 128:(k + 1) * 128])(hT)
                            import os as _os
                            _no_inj = bool(_os.environ.get("KERNEL_TAIL_NO_INJECT"))
                            # half 1 first: the next wave's matmuls consume
                            # k=4..7 (WH_ORDER) before k=0..3, so produce the
                            # matching h^T blocks first
                            for half in (1, 0):
                                if _no_inj:
                                    pt0 = mm_half(lhsT_of_k, wh_rhs, half,
                                                  k_order=WH_ORDER)
                                    pt = []
                                    for g in range(4):
                                        col0 = g * H + half * 512
                                        zg = work.tile([128, 512], DT.float32,
                                                       tag=f"zg{g}",
                                                       name=f"zg{g}", bufs=2)
                                        nc.vector.tensor_add(
                                            zg[:], pt0[g][:],
                                            zx_sb[:, col0:col0 + 512])
                                        pt.append(zg)
                                else:
                                    pt = mm_half(lhsT_of_k, wh_rhs, half,
                                                 inject=zx_sb, k_order=WH_ORDER)
                                blk = slice(half * 512, half * 512 + 512)
                                nc.scalar.activation(gi[:, blk], pt[0][:],
                                                     AF.Sigmoid)
                                nc.scalar.activation(gg[:, blk], pt[2][:],
                                                     AF.Tanh)
                                m1b = work.tile([128, 512], DT.float32,
                                                tag="m1b", name="m1b", bufs=2)
                                nc.vector.tensor_mul(m1b[:], gi[:, blk],
                                                     gg[:, blk])
                                nc.scalar.activation(gf[:, blk], pt[1][:],
                                                     AF.Sigmoid)
                                t1b = work.tile([128, 512], DT.float32,
                                                tag="t1b", name="t1b", bufs=2)
                                nc.vector.tensor_mul(t1b[:], gf[:, blk],
                                                     prev_ncv[:, blk])
                                nc.vector.tensor_add(ncv[:, blk], t1b[:],
                                                     m1b[:])
                                nc.scalar.activation(go[:, blk], pt[3][:],
                                                     AF.Sigmoid)
                                tncb = work.tile([128, 512], DT.float32,
                                                 tag="tncb", name="tncb",
                                                 bufs=2)
                                nc.scalar.activation(tncb[:], ncv[:, blk],
                                                     AF.Tanh)
                                nc.vector.tensor_mul(nh[:, blk], go[:, blk],
                                                     tncb[:])
                                nc.vector.tensor_copy(nhb[:, blk], nh[:, blk])
                                if build_next:
                                    if _no_xbar:
                                        for kk in range(half * 4, half * 4 + 4):
                                            bk = slice(kk * 128, (kk + 1) * 128)
                                            nc.sync.dma_start(
                                                out=hT_next[:, bk],
                                                in_=nhb[:, bk].rearrange(
                                                    "a b -> b a"))
                                    else:
                                        # blocked transpose of this half's 4
                                        # k-chunks in ONE xbar DMA
                                        nc.sync.dma_start_transpose(
                                            out=hT_next.rearrange(
                                                "p (j c) -> p j c", c=128)
                                            [:, half * 4:half * 4 + 4, :],
                                            in_=nhb[:, blk])

                            nc.gpsimd.dma_start(
                                out=y_ap[int(P[d]):int(P[d]) + 128, :],
                                in_=nh[:])
                            prev_ncv, prev_nhb = ncv, nhb
                            pending_hT = hT_next if build_next else None
                            continue

                        # ---- DRAM-path wave ----
                        pending_hT = None
                        if d == 0:
                            hT = None  # allocated per m-tile below
                        else:
                            hT = hTP.tile([128, hT_cols_max], DT.bfloat16,
                                          tag="hT", name="hT")
                            prev = int(P[d - 1])
                            nc.sync.dma_start_transpose(
                                out=hT[:, 0:KT * ncols].rearrange(
                                    "p (j m) -> p j m", m=ncols),
                                in_=hs_ap[prev:prev + ncols, :])

                        for mt in range(Md):
                            zx_sb = zxP.tile([128, G], DT.bfloat16, tag="zx",
                                             name="zx_sb")
                            r = int(V[d]) + mt * 128 - zx_row0
                            nc.gpsimd.dma_start(out=zx_sb[:],
                                                in_=zx_ap[r:r + 128, :])

                            if d == 0:
                                c_src = cP.tile([128, H], DT.float32, tag="c",
                                                name="c_sb")
                                nc.gpsimd.dma_start(
                                    out=c_src[:],
                                    in_=c0_d.ap()[mt * 128:mt * 128 + 128, :])
                            else:
                                c_src = cP.tile([128, H], DT.float32, tag="c",
                                                name="c_sb")
                                prev = int(P[d - 1])
                                nc.gpsimd.dma_start(
                                    out=c_src[:],
                                    in_=cs_ap[prev + mt * 128:
                                              prev + mt * 128 + 128, :])

                            if d == 0:
                                hT = hTP.tile([128, KT * 128], DT.bfloat16,
                                              tag="hT0", name="hT0")
                                for k in range(KT):
                                    nc.gpsimd.dma_start(
                                        out=hT[:, k * 128:(k + 1) * 128],
                                        in_=ht0_d.ap()[k][:, mt * 128:
                                                          mt * 128 + 128])
                                lhsT_of_k = (lambda hh: lambda k:
                                             hh[:, k * 128:(k + 1) * 128])(hT)
                            else:
                                lhsT_of_k = (lambda hh, nn, mm: lambda k:
                                             hh[:, k * nn + mm * 128:
                                                k * nn + mm * 128 + 128])(
                                    hT, ncols, mt)

                            z_sb = work.tile([128, G], DT.bfloat16, tag="zws",
                                             name="z_sb")
                            for half in range(2):
                                pt = mm_half(lhsT_of_k, wh_rhs, half,
                                             k_order=WH_ORDER)
                                for g in range(4):
                                    col0 = g * H + half * 512
                                    nc.vector.tensor_add(
                                        z_sb[:, col0:col0 + 512], pt[g][:],
                                        zx_sb[:, col0:col0 + 512])

                            nxt_sbuf = (d + 1 < D) and sbuf_in[d + 1]
                            nwave = int(M[d + 1]) * 128 if d + 1 < D else 0
                            ncv, nhb = gate_math(
                                z_sb, c_src, int(P[d]) + mt * 128,
                                store_h=(d + 1 < D and not nxt_sbuf
                                         and mt * 128 < nwave),
                                store_c=(d + 1 < D and not nxt_sbuf
                                         and mt * 128 < nwave))
                            if mt == 0:
                                prev_ncv, prev_nhb = ncv, nhb

    nc.compile()
    return nc


# ---------------------------------------------------------------------------
# Entry point
# ---------------------------------------------------------------------------

_PROGRAM_CACHE = {}


def _run(inputs, trace=False):
    prep = _prep_inputs(**inputs)
    sch = prep["sch"]
    D, U, M, V, P = sch["D"], sch["U"], sch["M"], sch["V"], sch["P"]

    in_maps = []
    for c in range(NCORES):
        m = {
            "xt": prep["xt_blocks"][c],
            "wi": prep["Wi_l"],
            "wh": prep["Wh_l"],
            "bbc": prep["bbc"],
            "ident": prep["ident"],
        }
        if not prep["zero_init"]:
            m["ht0"] = prep["ht0_blocks"][c]
            m["c0"] = prep["c0_blocks"][c]
        in_maps.append(m)

    # Retry ladder: rare transient device errors have been observed on the
    # shared terminal; retry twice, then once more with the conservative
    # (no SBUF-resident tail waves) program variant.
    import time as _time
    res = None
    last_err = None
    for attempt, no_tail in enumerate([False, False, True]):
        key = (D, tuple(M.tolist()), tuple(U.tolist()), prep["Mzx"],
               prep["zx_row0"], prep["zero_init"], no_tail)
        try:
            if key not in _PROGRAM_CACHE:
                _PROGRAM_CACHE[key] = _build_program(
                    D, U, M, V, P, prep["Mzx"], prep["zx_row0"],
                    prep["zx_start_tile"], prep["zero_init"], no_tail=no_tail)
            nc = _PROGRAM_CACHE[key]
            res = run_bass_kernel_spmd(nc, in_maps,
                                       core_ids=list(range(NCORES)),
                                       trace=trace)
            break
        except Exception as e:  # noqa: BLE001 - retry on device hiccups
            last_err = e
            sys.stderr.write(f"kernel attempt {attempt} failed: {e!r}\n")
            trace = False  # profiling hook may be wedged; drop it on retry
            _time.sleep(2.0)
    if res is None:
        raise last_err

    T, B = prep["T"], prep["B"]
    y_full = np.empty((T * B, H), np.float32)
    core_pos = prep["core_pos"]; padded_row = prep["padded_row"]
    for c in range(NCORES):
        sel = core_pos == c
        y_full[sel] = res.results[c]["y"][padded_row[sel]]
    return y_full, res


def kernel(**inputs) -> np.ndarray:
    y, _ = _run(inputs, trace=False)
    return y



# revision 8
# speedup vs baseline: 1.0690x; 1.0690x over previous
"""Trainium2 Bass kernel for the 2-player masked LSTM scan.

Reference semantics (T=128 steps, B=256 batch, C=1024 in, H=1024 hidden):
  per step t, batch b: the active player's (c,h) (selected by main[t,b]) runs
  one LSTM cell z = x@Wi + h@Wh + b with fused i,f,g,o gates; the result is
  written back only to the active player's state, and both players' states are
  zeroed where done[t,b].

Algorithmic structure: done/main are inputs, so the dependency structure is
known on the host.  Each (b, segment, player) triple is an independent chain;
position depth d depends only on depth d-1.  Chains sorted by length and
round-robined over the 8 cores turn the scan into D~9 dense waves.

Device schedule (zero-init fast path):
 - phase A computes zx = x@Wi + b for all depth>=1 positions (the small "zx
   tiles", emitted FIRST so every wave's zx is ready early), then the depth-0
   positions fused with their full gate math ("w0 tiles"; c == 0 there, so the
   f-gate matmuls are skipped entirely -- 25% less PE work on those tiles).
 - phase B waves run h@Wh in fp8 e4m3 with DoubleRow perf mode (2 k-subtiles
   per matmul, 2x PE throughput; rel-err contribution measured ~0.004), the
   bf16 zx injected into PSUM via an identity matmul so the ACT engine reads
   gates straight from PSUM.  Wave jobs are interleaved into the phase-A tile
   stream: single-m-tile tail waves get a w0 "filler" tile emitted after them
   so the PE stays busy during the serial gate-math/transpose latency.
 - inter-wave state: h^T via per-half xbar transposes (bf16) + fp8 cast into
   per-wave SBUF tiles; c stays in SBUF for single-tile consumers and round-
   trips DRAM for the big early waves.
"""

import sys

sys.path.insert(0, "/opt/trn_rl_repo")

import numpy as np
import ml_dtypes

import concourse.bass as bass
import concourse.tile as tile
from concourse import bacc, mybir
from concourse.bass_utils import run_bass_kernel_spmd

BF16 = ml_dtypes.bfloat16
F8 = ml_dtypes.float8_e4m3
AF = mybir.ActivationFunctionType
DT = mybir.dt
DR = mybir.MatmulPerfMode.DoubleRow

NCORES = 8
H = 1024
CIN = 1024
G = 4 * H
KT = CIN // 128  # 8 k-tiles for both contractions


# ---------------------------------------------------------------------------
# Host-side schedule construction
# ---------------------------------------------------------------------------

def _build_schedule(done, main, T, B):
    """Chain decomposition of the (t, b) grid."""
    done2 = done.reshape(T, B).astype(bool)
    main2 = main.reshape(T, B).astype(bool)

    seg = np.zeros((T, B), np.int64)
    if T > 1:
        seg[1:] = np.cumsum(done2[:-1], axis=0)
    player = main2.astype(np.int64)
    key = (np.arange(B)[None, :] * (T + 1) + seg) * 2 + player  # [T, B]
    flat_key = key.reshape(-1)  # position p = t*B + b
    order = np.argsort(flat_key, kind="stable")  # chain-major, t-ascending
    sorted_keys = flat_key[order]
    uk, first_idx, inv = np.unique(sorted_keys, return_index=True, return_inverse=True)
    chain_len = np.diff(np.append(first_idx, len(sorted_keys)))
    npos = T * B

    depth = np.empty(npos, np.int64)
    depth[order] = np.arange(npos) - first_idx[inv]
    chain_id = np.empty(npos, np.int64)
    chain_id[order] = inv

    n_chains = len(uk)
    chain_b = uk // (2 * (T + 1))
    chain_seg = (uk // 2) % (T + 1)
    chain_player = uk % 2

    chain_order = np.argsort(-chain_len, kind="stable")
    rank_of_chain = np.empty(n_chains, np.int64)
    rank_of_chain[chain_order] = np.arange(n_chains)
    core_of_chain = (rank_of_chain % NCORES).astype(np.int64)
    core_rank = rank_of_chain // NCORES

    D = int(chain_len.max())
    lens_sorted = np.sort(chain_len)
    N_d = np.array([n_chains - np.searchsorted(lens_sorted, d, side="right")
                    for d in range(D)], np.int64)
    U = np.ceil(N_d / NCORES).astype(np.int64)      # uniform per-core wave rows
    M = np.ceil(U / 128).astype(np.int64)           # padded wave m-tiles
    V = np.concatenate([[0], np.cumsum(U)])          # packed row offsets
    P = np.concatenate([[0], np.cumsum(M * 128)])    # padded row offsets

    return dict(
        depth=depth, chain_id=chain_id, core_of_chain=core_of_chain,
        core_rank=core_rank, chain_b=chain_b, chain_seg=chain_seg,
        chain_player=chain_player, D=D, U=U, M=M, V=V, P=P,
    )


def _zx_tiles(D, M, V, zero_init):
    """zx-region m-tile count: covers every wave tile's padded [128-row read."""
    dz = 1 if zero_init else 0
    if D <= dz:
        return 0
    need = int(V[D] - V[dz])
    for d in range(dz, D):
        need = max(need, int(V[d] - V[dz]) + int(M[d]) * 128)
    return (need + 127) // 128


def _prep_inputs(x, c1, h1, c2, h2, Wi, Wh, b, done, main):
    """Build per-core device input arrays + output scatter indices."""
    B = c1.shape[0]
    T = x.shape[0] // B
    sch = _build_schedule(np.asarray(done), np.asarray(main), T, B)
    D, U, M, V, P = sch["D"], sch["U"], sch["M"], sch["V"], sch["P"]

    zero_init = not (np.any(c1) or np.any(h1) or np.any(c2) or np.any(h2))
    dz = 1 if zero_init else 0
    Zt = _zx_tiles(D, M, V, zero_init)
    M0 = int(M[0])
    NT = Zt + (M0 if zero_init else 0)

    depth = sch["depth"]
    chain_id = sch["chain_id"]
    core_pos = sch["core_of_chain"][chain_id]
    rank = sch["core_rank"][chain_id]
    if zero_init:
        xrow = np.where(depth == 0, Zt * 128 + rank,
                        (V[np.minimum(depth, D)] - V[dz]) + rank)
    else:
        xrow = V[depth] + rank
    padded_row = P[depth] + rank

    x = np.ascontiguousarray(np.asarray(x, np.float32))
    xt_blocks = []
    for c in range(NCORES):
        sel = core_pos == c
        Xp = np.zeros((NT * 128, CIN), np.float32)
        Xp[xrow[sel]] = x[sel]
        # lhsT block layout: [mt, p, k*128 + m] = Xp[mt*128+m, k*128+p]
        xt = Xp.reshape(NT, 128, KT, 128).transpose(0, 3, 2, 1).reshape(NT, 128, CIN)
        xt_blocks.append(np.ascontiguousarray(xt.astype(BF16)))

    Wi_l = np.ascontiguousarray(
        np.asarray(Wi, np.float32).reshape(KT, 128, G).astype(BF16))
    Wh8 = np.ascontiguousarray(
        np.asarray(Wh, np.float32).reshape(KT, 128, G).astype(F8))
    bbc = np.ascontiguousarray(
        np.broadcast_to(np.asarray(b, np.float32)[None, :], (128, G)).astype(BF16))
    ident = np.ascontiguousarray(np.eye(128, dtype=np.float32).astype(BF16))

    ht0_blocks = [None] * NCORES
    c0_blocks = [None] * NCORES
    if not zero_init:
        h1 = np.asarray(h1, np.float32); h2 = np.asarray(h2, np.float32)
        c1 = np.asarray(c1, np.float32); c2 = np.asarray(c2, np.float32)
        hin = np.where(sch["chain_player"][:, None] > 0, h1[sch["chain_b"]],
                       h2[sch["chain_b"]])
        cin_ = np.where(sch["chain_player"][:, None] > 0, c1[sch["chain_b"]],
                        c2[sch["chain_b"]])
        live = sch["chain_seg"] == 0
        hin = np.where(live[:, None], hin, 0.0)
        cin_ = np.where(live[:, None], cin_, 0.0)
        for c in range(NCORES):
            selc = sch["core_of_chain"] == c
            rows = sch["core_rank"][selc]
            Hp = np.zeros((M0 * 128, H), np.float32)
            Cp = np.zeros((M0 * 128, H), np.float32)
            Hp[rows] = hin[selc]
            Cp[rows] = cin_[selc]
            # transposed fp8 layout for lhsT: [p, k, col] = Hp[col, k*128+p]
            ht0 = Hp.reshape(M0 * 128, KT, 128).transpose(2, 1, 0)
            ht0_blocks[c] = np.ascontiguousarray(ht0.astype(F8))
            c0_blocks[c] = np.ascontiguousarray(Cp)

    return dict(
        sch=sch, zero_init=zero_init, Zt=Zt, NT=NT,
        xt_blocks=xt_blocks, Wi_l=Wi_l, Wh8=Wh8, bbc=bbc, ident=ident,
        ht0_blocks=ht0_blocks, c0_blocks=c0_blocks,
        core_pos=core_pos, padded_row=padded_row, T=T, B=B,
    )


# ---------------------------------------------------------------------------
# Device program
# ---------------------------------------------------------------------------

def _build_program(D, U, M, V, P, zero_init):
    nc = bacc.Bacc("TRN2", target_bir_lowering=False, debug=False)

    M_ = [int(m) for m in M]
    P_ = [int(p) for p in P]
    dz = 1 if zero_init else 0
    M0 = M_[0]
    Zt = _zx_tiles(D, M, V, zero_init)
    NT = Zt + (M0 if zero_init else 0)
    Ptot = P_[D]
    Vz = [int(V[d] - V[dz]) for d in range(D + 1)]
    have_waves = D > dz

    xt_d = nc.dram_tensor("xt", [NT, 128, CIN], DT.bfloat16, kind="ExternalInput")
    wi_d = nc.dram_tensor("wi", [KT, 128, G], DT.bfloat16, kind="ExternalInput")
    bbc_d = nc.dram_tensor("bbc", [128, G], DT.bfloat16, kind="ExternalInput")
    id_d = nc.dram_tensor("ident", [128, 128], DT.bfloat16, kind="ExternalInput")
    y_d = nc.dram_tensor("y", [Ptot, H], DT.float32, kind="ExternalOutput")
    if have_waves:
        wh_d = nc.dram_tensor("wh8", [KT, 128, G], DT.float8e4,
                              kind="ExternalInput")
        zx_d = nc.dram_tensor("zx", [Zt * 128, 2, 2048], DT.bfloat16,
                              kind="Internal")
        zx_ap = zx_d.ap()
    # c-state DRAM regions for waves whose consumer wave has >1 m-tile
    cs_off = {}
    acc = 0
    for d in range(D - 1):
        if M_[d + 1] > 1:
            cs_off[d] = acc
            acc += M_[d + 1] * 128
    if acc:
        cs_d = nc.dram_tensor("cstate", [acc, H], DT.float32, kind="Internal")
        cs_ap = cs_d.ap()
    if not zero_init:
        ht0_d = nc.dram_tensor("ht0", [128, KT, M0 * 128], DT.float8e4,
                               kind="ExternalInput")
        c0_d = nc.dram_tensor("c0", [M0 * 128, H], DT.float32,
                              kind="ExternalInput")

    xt_ap = xt_d.ap()
    y_ap = y_d.ap()

    with tile.TileContext(nc) as tc:
        from contextlib import ExitStack
        with ExitStack() as es:
            const = es.enter_context(tc.tile_pool(name="const", bufs=1))
            wp = es.enter_context(tc.tile_pool(name="weights", bufs=1))
            xtP = es.enter_context(tc.tile_pool(name="xtP", bufs=3))
            zbP = es.enter_context(tc.tile_pool(name="zbP", bufs=2))
            zxP = es.enter_context(tc.tile_pool(name="zxP", bufs=3))
            gP = es.enter_context(tc.tile_pool(name="gP", bufs=1))
            fP = es.enter_context(tc.tile_pool(name="fP", bufs=2))
            sP = es.enter_context(tc.tile_pool(name="sP", bufs=2))
            hTP = es.enter_context(tc.tile_pool(name="hTP", bufs=1))
            cP = es.enter_context(tc.tile_pool(name="cP", bufs=2))
            psum = es.enter_context(tc.tile_pool(name="psum", bufs=2,
                                                 space="PSUM"))

            # ---- weights / constants (emission order = DMA priority) ----
            wi_sb = wp.tile([128, KT, G], DT.bfloat16, tag="wi")
            # k=0 on the gpsimd queue so the first matmul unblocks early;
            # the rest on the sync queue to load in parallel with xt tiles.
            nc.gpsimd.dma_start(out=wi_sb[:, 0:1, :], in_=wi_d.ap()[0])
            for k in range(1, KT):
                nc.sync.dma_start(out=wi_sb[:, k:k + 1, :], in_=wi_d.ap()[k])
            if have_waves:
                wh8_sb = wp.tile([128, KT, G], DT.float8e4, tag="wh8")
                for k in range(KT):
                    nc.sync.dma_start(out=wh8_sb[:, k:k + 1, :],
                                      in_=wh_d.ap()[k])
            ident_sb = const.tile([128, 128], DT.bfloat16, tag="ident")
            bbc_sb = const.tile([128, G], DT.bfloat16, tag="bbc")

            hT8 = {}
            for d in range(1, D):
                hT8[d] = hTP.tile([128, KT, M_[d] * 128], DT.float8e4,
                                  tag=f"hT{d}", name=f"hT{d}")
            if not zero_init:
                ht0_sb = wp.tile([128, KT, M0 * 128], DT.float8e4, tag="ht0")
                nc.gpsimd.dma_start(out=ht0_sb[:], in_=ht0_d.ap()[:])

            # gate tiles
            gi = gP.tile([128, H], DT.bfloat16, tag="gi", name="gi")
            gf = gP.tile([128, H], DT.bfloat16, tag="gf", name="gf")
            gg = gP.tile([128, H], DT.bfloat16, tag="gg", name="gg")
            go = gP.tile([128, H], DT.bfloat16, tag="go", name="go")

            keep_state = {}  # wave d -> SBUF fp32 c tile (single-tile consumer)

            def mm_phaseA(xt_sb, half, skip_f):
                pt = {}
                for g in range(4):
                    if skip_f and g == 1:
                        continue
                    pt[g] = psum.tile([128, 512], DT.float32, tag=f"ph{g}",
                                      name=f"ph{g}")
                for k in range(KT):
                    for g in pt:
                        col0 = g * H + half * 512
                        nc.tensor.matmul(
                            pt[g][:], lhsT=xt_sb[:, k:k + 1, :],
                            rhs=wi_sb[:, k:k + 1, col0:col0 + 512],
                            start=(k == 0), stop=(k == KT - 1),
                            skip_group_check=True)
                return pt

            def drain(pt, half):
                """PSUM -> SBUF bf16 with bias add (DVE; GPSIMD can't read PSUM)."""
                zbs = {}
                for g, p in pt.items():
                    col0 = g * H + half * 512
                    zb = zbP.tile([128, 512], DT.bfloat16, tag=f"zb{g}",
                                  name=f"zb{g}")
                    nc.vector.tensor_add(zb[:], p[:], bbc_sb[:, col0:col0 + 512])
                    zbs[g] = zb
                return zbs

            def zx_job(t):
                xt_sb = xtP.tile([128, KT, 128], DT.bfloat16, tag="xt",
                                 name="xt")
                nc.gpsimd.dma_start(out=xt_sb[:], in_=xt_ap[t])
                if t == 0:
                    nc.gpsimd.dma_start(out=bbc_sb[:], in_=bbc_d.ap()[:])
                    nc.gpsimd.dma_start(out=ident_sb[:], in_=id_d.ap()[:])
                for half in range(2):
                    pt = mm_phaseA(xt_sb, half, skip_f=False)
                    zbs = drain(pt, half)
                    for g in range(4):
                        nc.gpsimd.dma_start(
                            out=zx_ap[t * 128:(t + 1) * 128, half:half + 1,
                                      g * 512:(g + 1) * 512],
                            in_=zbs[g][:])

            def w0_job(j):
                """Fused depth-0 tile (zero-init: c == 0, f gate skipped)."""
                feeds = have_waves and j < M_[1]
                t = Zt + j
                xt_sb = xtP.tile([128, KT, 128], DT.bfloat16, tag="xt",
                                 name="xt")
                nc.gpsimd.dma_start(out=xt_sb[:], in_=xt_ap[t])
                if Zt == 0 and j == 0:
                    nc.gpsimd.dma_start(out=bbc_sb[:], in_=bbc_d.ap()[:])
                    nc.gpsimd.dma_start(out=ident_sb[:], in_=id_d.ap()[:])

                if feeds and M_[1] == 1:
                    ncv = sP.tile([128, H], DT.float32, tag="keepT",
                                  name="ncv_keep", bufs=2)
                    keep_state[0] = ncv
                else:
                    ncv = sP.tile([128, H], DT.float32, tag="ncv", name="ncv")
                nh = sP.tile([128, H], DT.float32, tag="nh", name="nh")
                if feeds:
                    nhb = sP.tile([128, H], DT.bfloat16, tag="nhb", name="nhb")
                    hTb = sP.tile([128, KT, 128], DT.bfloat16, tag="hTb",
                                  name="hTb")
                for half in range(2):
                    blk = slice(half * 512, half * 512 + 512)
                    pt = mm_phaseA(xt_sb, half, skip_f=True)
                    zbs = drain(pt, half)
                    nc.scalar.activation(gi[:, blk], zbs[0][:], AF.Sigmoid)
                    nc.scalar.activation(gg[:, blk], zbs[2][:], AF.Tanh)
                    nc.gpsimd.tensor_mul(ncv[:, blk], gi[:, blk], gg[:, blk])
                    nc.scalar.activation(go[:, blk], zbs[3][:], AF.Sigmoid)
                    tancb = fP.tile([128, 512], DT.float32, tag="tanc",
                                    name="tanc")
                    nc.scalar.activation(tancb[:], ncv[:, blk], AF.Tanh)
                    nc.vector.tensor_mul(nh[:, blk], go[:, blk], tancb[:])
                    if feeds:
                        nc.gpsimd.tensor_copy(nhb[:, blk], nh[:, blk])
                        nc.sync.dma_start_transpose(
                            out=hTb[:, half * 4:(half + 1) * 4, :],
                            in_=nhb[:, blk])
                        nc.gpsimd.tensor_copy(
                            hT8[1][:, half * 4:(half + 1) * 4,
                                   j * 128:(j + 1) * 128],
                            hTb[:, half * 4:(half + 1) * 4, :])
                nc.gpsimd.dma_start(out=y_ap[j * 128:(j + 1) * 128, :],
                                    in_=nh[:])
                if feeds and M_[1] > 1:
                    nc.gpsimd.dma_start(
                        out=cs_ap[cs_off[0] + j * 128:cs_off[0] + (j + 1) * 128, :],
                        in_=ncv[:])

            def wv_job(d, j):
                feeds = (d + 1 < D) and (j < M_[d + 1])
                # zx prefetch (both halves)
                zxh = []
                for half in range(2):
                    zt = zxP.tile([128, 2048], DT.bfloat16, tag="zxh",
                                  name="zxh")
                    nc.gpsimd.dma_start(
                        out=zt[:],
                        in_=zx_ap[Vz[d] + j * 128:Vz[d] + (j + 1) * 128,
                                  half:half + 1, :])
                    zxh.append(zt)
                # c source
                if d == 0 and not zero_init:
                    c_src = cP.tile([128, H], DT.float32, tag="c", name="c_sb")
                    nc.gpsimd.dma_start(
                        out=c_src[:], in_=c0_d.ap()[j * 128:(j + 1) * 128, :])
                elif M_[d] == 1:
                    c_src = keep_state[d - 1]
                else:
                    c_src = cP.tile([128, H], DT.float32, tag="c", name="c_sb")
                    nc.gpsimd.dma_start(
                        out=c_src[:],
                        in_=cs_ap[cs_off[d - 1] + j * 128:
                                  cs_off[d - 1] + (j + 1) * 128, :])
                # lhsT source
                hsrc = ht0_sb if (d == 0 and not zero_init) else hT8[d]

                if feeds and M_[d + 1] == 1:
                    ncv = sP.tile([128, H], DT.float32, tag="keepT",
                                  name="ncv_keep", bufs=2)
                    keep_state[d] = ncv
                else:
                    ncv = sP.tile([128, H], DT.float32, tag="ncv", name="ncv")
                nh = sP.tile([128, H], DT.float32, tag="nh", name="nh")
                if feeds:
                    nhb = sP.tile([128, H], DT.bfloat16, tag="nhb", name="nhb")
                    hTb = sP.tile([128, KT, 128], DT.bfloat16, tag="hTb",
                                  name="hTb")
                for half in range(2):
                    blk = slice(half * 512, half * 512 + 512)
                    pt = []
                    for g in range(4):
                        pt.append(psum.tile([128, 512], DT.float32,
                                            tag=f"ph{g}", name=f"ph{g}"))
                    for kp in range(KT // 2):
                        for g in range(4):
                            col0 = g * H + half * 512
                            nc.tensor.matmul(
                                pt[g][:],
                                lhsT=hsrc[:, 2 * kp:2 * kp + 2,
                                          j * 128:(j + 1) * 128],
                                rhs=wh8_sb[:, 2 * kp:2 * kp + 2,
                                           col0:col0 + 512],
                                start=(kp == 0), stop=False,
                                perf_mode=DR, skip_group_check=True)
                    for g in range(4):
                        nc.tensor.matmul(
                            pt[g][:], lhsT=ident_sb[:],
                            rhs=zxh[half][:, g * 512:(g + 1) * 512],
                            start=False, stop=True, skip_group_check=True)
                    # gates straight from PSUM (zx inject carries the bias)
                    nc.scalar.activation(gi[:, blk], pt[0][:], AF.Sigmoid)
                    nc.scalar.activation(gg[:, blk], pt[2][:], AF.Tanh)
                    m1b = fP.tile([128, 512], DT.float32, tag="m1", name="m1")
                    nc.vector.tensor_mul(m1b[:], gi[:, blk], gg[:, blk])
                    nc.scalar.activation(gf[:, blk], pt[1][:], AF.Sigmoid)
                    t1b = fP.tile([128, 512], DT.float32, tag="t1", name="t1")
                    nc.gpsimd.tensor_mul(t1b[:], gf[:, blk], c_src[:, blk])
                    nc.vector.tensor_add(ncv[:, blk], t1b[:], m1b[:])
                    nc.scalar.activation(go[:, blk], pt[3][:], AF.Sigmoid)
                    tancb = fP.tile([128, 512], DT.float32, tag="tanc",
                                    name="tanc")
                    nc.scalar.activation(tancb[:], ncv[:, blk], AF.Tanh)
                    nc.vector.tensor_mul(nh[:, blk], go[:, blk], tancb[:])
                    if feeds:
                        nc.gpsimd.tensor_copy(nhb[:, blk], nh[:, blk])
                        nc.sync.dma_start_transpose(
                            out=hTb[:, half * 4:(half + 1) * 4, :],
                            in_=nhb[:, blk])
                        nc.gpsimd.tensor_copy(
                            hT8[d + 1][:, half * 4:(half + 1) * 4,
                                       j * 128:(j + 1) * 128],
                            hTb[:, half * 4:(half + 1) * 4, :])
                nc.gpsimd.dma_start(
                    out=y_ap[P_[d] + j * 128:P_[d] + (j + 1) * 128, :],
                    in_=nh[:])
                if feeds and M_[d + 1] > 1:
                    nc.gpsimd.dma_start(
                        out=cs_ap[cs_off[d] + j * 128:
                                  cs_off[d] + (j + 1) * 128, :],
                        in_=ncv[:])

            # ---- job order ----
            jobs = []
            if zero_init:
                jobs += [("zx", t) for t in range(Zt)]
                head = min(M_[1], M0) if have_waves else 0
                jobs += [("w0", j) for j in range(head)]
                from collections import deque
                fill = deque(range(head, M0))
                for d in range(1, D):
                    for j in range(M_[d]):
                        jobs.append(("wv", d, j))
                    if d + 1 < D and M_[d] == 1 and fill:
                        jobs.append(("w0", fill.popleft()))
                while fill:
                    jobs.append(("w0", fill.popleft()))
            else:
                jobs += [("zx", t) for t in range(Zt)]
                for d in range(D):
                    for j in range(M_[d]):
                        jobs.append(("wv", d, j))

            for job in jobs:
                if job[0] == "zx":
                    zx_job(job[1])
                elif job[0] == "w0":
                    w0_job(job[1])
                else:
                    wv_job(job[1], job[2])

    nc.compile()
    return nc


# ---------------------------------------------------------------------------
# Entry point
# ---------------------------------------------------------------------------

_PROGRAM_CACHE = {}


def _run(inputs, trace=False):
    prep = _prep_inputs(**inputs)
    sch = prep["sch"]
    D, U, M, V, P = sch["D"], sch["U"], sch["M"], sch["V"], sch["P"]

    in_maps = []
    for c in range(NCORES):
        m = {
            "xt": prep["xt_blocks"][c],
            "wi": prep["Wi_l"],
            "bbc": prep["bbc"],
            "ident": prep["ident"],
        }
        if D > (1 if prep["zero_init"] else 0):
            m["wh8"] = prep["Wh8"]
        if not prep["zero_init"]:
            m["ht0"] = prep["ht0_blocks"][c]
            m["c0"] = prep["c0_blocks"][c]
        in_maps.append(m)

    # Retry ladder: rare transient device errors have been observed on the
    # shared terminal; retry a couple of times.
    import time as _time
    res = None
    last_err = None
    for attempt in range(3):
        key = (D, tuple(int(v) for v in M), tuple(int(v) for v in U),
               prep["zero_init"])
        try:
            if key not in _PROGRAM_CACHE:
                _PROGRAM_CACHE[key] = _build_program(
                    D, U, M, V, P, prep["zero_init"])
            nc = _PROGRAM_CACHE[key]
            res = run_bass_kernel_spmd(nc, in_maps,
                                       core_ids=list(range(NCORES)),
                                       trace=trace)
            break
        except Exception as e:  # noqa: BLE001 - retry on device hiccups
            last_err = e
            sys.stderr.write(f"kernel attempt {attempt} failed: {e!r}\n")
            trace = False  # profiling hook may be wedged; drop it on retry
            _time.sleep(2.0)
    if res is None:
        raise last_err

    T, B = prep["T"], prep["B"]
    y_full = np.empty((T * B, H), np.float32)
    core_pos = prep["core_pos"]
    padded_row = prep["padded_row"]
    for c in range(NCORES):
        sel = core_pos == c
        y_full[sel] = res.results[c]["y"][padded_row[sel]]
    return y_full, res


def kernel(**inputs) -> np.ndarray:
    y, _ = _run(inputs, trace=False)
    return y


# revision 16
# speedup vs baseline: 1.2299x; 1.1505x over previous
"""Trainium2 Bass kernel for the 2-player masked LSTM scan.

Reference semantics (T=128 steps, B=256 batch, C=1024 in, H=1024 hidden):
  per step t, batch b: the active player's (c,h) (selected by main[t,b]) runs
  one LSTM cell z = x@Wi + h@Wh + b with fused i,f,g,o gates; the result is
  written back only to the active player's state, and both players' states are
  zeroed where done[t,b].

Algorithmic structure: done/main are inputs, so the dependency structure is
known on the host.  Each (b, segment, player) triple is an independent chain;
position depth d depends only on depth d-1.  Chains sorted by length and
round-robined over the 8 cores turn the scan into D~9 dense waves.

Device schedule (zero-init fast path):
 - phase A computes zx = x@Wi + b for all depth>=1 positions (the small "zx
   tiles", emitted FIRST so every wave's zx is ready early), then the depth-0
   positions fused with their full gate math ("w0 tiles"; c == 0 there, so the
   f-gate matmuls are skipped entirely -- 25% less PE work on those tiles).
 - phase B waves run h@Wh in fp8 e4m3 with DoubleRow perf mode (2 k-subtiles
   per matmul, 2x PE throughput; rel-err contribution measured ~0.004), the
   bf16 zx injected into PSUM via an identity matmul so the ACT engine reads
   gates straight from PSUM.  Wave jobs are interleaved into the phase-A tile
   stream: single-m-tile tail waves get a w0 "filler" tile emitted after them
   so the PE stays busy during the serial gate-math/transpose latency.
 - inter-wave state: h^T via per-half xbar transposes (bf16) + fp8 cast into
   per-wave SBUF tiles; c stays in SBUF for single-tile consumers and round-
   trips DRAM for the big early waves.
"""

import sys

sys.path.insert(0, "/opt/trn_rl_repo")

import numpy as np
import ml_dtypes

import concourse.bass as bass
import concourse.tile as tile
from concourse import bacc, mybir
from concourse.bass_utils import run_bass_kernel_spmd

BF16 = ml_dtypes.bfloat16
F8 = ml_dtypes.float8_e4m3
AF = mybir.ActivationFunctionType
DT = mybir.dt
DR = mybir.MatmulPerfMode.DoubleRow

NCORES = 8
H = 1024
CIN = 1024
G = 4 * H
KT = CIN // 128  # 8 k-tiles for both contractions


# ---------------------------------------------------------------------------
# Host-side schedule construction
# ---------------------------------------------------------------------------

def _build_schedule(done, main, T, B):
    """Chain decomposition of the (t, b) grid."""
    done2 = done.reshape(T, B).astype(bool)
    main2 = main.reshape(T, B).astype(bool)

    seg = np.zeros((T, B), np.int64)
    if T > 1:
        seg[1:] = np.cumsum(done2[:-1], axis=0)
    player = main2.astype(np.int64)
    key = (np.arange(B)[None, :] * (T + 1) + seg) * 2 + player  # [T, B]
    flat_key = key.reshape(-1)  # position p = t*B + b
    order = np.argsort(flat_key, kind="stable")  # chain-major, t-ascending
    sorted_keys = flat_key[order]
    uk, first_idx, inv = np.unique(sorted_keys, return_index=True, return_inverse=True)
    chain_len = np.diff(np.append(first_idx, len(sorted_keys)))
    npos = T * B

    depth = np.empty(npos, np.int64)
    depth[order] = np.arange(npos) - first_idx[inv]
    chain_id = np.empty(npos, np.int64)
    chain_id[order] = inv

    n_chains = len(uk)
    chain_b = uk // (2 * (T + 1))
    chain_seg = (uk // 2) % (T + 1)
    chain_player = uk % 2

    chain_order = np.argsort(-chain_len, kind="stable")
    rank_of_chain = np.empty(n_chains, np.int64)
    rank_of_chain[chain_order] = np.arange(n_chains)
    core_of_chain = (rank_of_chain % NCORES).astype(np.int64)
    core_rank = rank_of_chain // NCORES

    D = int(chain_len.max())
    lens_sorted = np.sort(chain_len)
    N_d = np.array([n_chains - np.searchsorted(lens_sorted, d, side="right")
                    for d in range(D)], np.int64)
    U = np.ceil(N_d / NCORES).astype(np.int64)      # uniform per-core wave rows
    M = np.ceil(U / 128).astype(np.int64)           # padded wave m-tiles
    V = np.concatenate([[0], np.cumsum(U)])          # packed row offsets
    P = np.concatenate([[0], np.cumsum(M * 128)])    # padded row offsets

    return dict(
        depth=depth, chain_id=chain_id, core_of_chain=core_of_chain,
        core_rank=core_rank, chain_b=chain_b, chain_seg=chain_seg,
        chain_player=chain_player, D=D, U=U, M=M, V=V, P=P,
    )


def _zx_tiles(D, M, V, zero_init):
    """zx-region m-tile count: covers every wave tile's padded [128-row read."""
    dz = 1 if zero_init else 0
    if D <= dz:
        return 0
    need = int(V[D] - V[dz])
    for d in range(dz, D):
        need = max(need, int(V[d] - V[dz]) + int(M[d]) * 128)
    return (need + 127) // 128


def _prep_inputs(x, c1, h1, c2, h2, Wi, Wh, b, done, main):
    """Build per-core device input arrays + output scatter indices."""
    B = c1.shape[0]
    T = x.shape[0] // B
    sch = _build_schedule(np.asarray(done), np.asarray(main), T, B)
    D, U, M, V, P = sch["D"], sch["U"], sch["M"], sch["V"], sch["P"]

    zero_init = not (np.any(c1) or np.any(h1) or np.any(c2) or np.any(h2))
    dz = 1 if zero_init else 0
    Zt = _zx_tiles(D, M, V, zero_init)
    M0 = int(M[0])
    NT = Zt + (M0 if zero_init else 0)

    depth = sch["depth"]
    chain_id = sch["chain_id"]
    core_pos = sch["core_of_chain"][chain_id]
    rank = sch["core_rank"][chain_id]
    if zero_init:
        xrow = np.where(depth == 0, Zt * 128 + rank,
                        (V[np.minimum(depth, D)] - V[dz]) + rank)
    else:
        xrow = V[depth] + rank
    padded_row = P[depth] + rank

    x = np.ascontiguousarray(np.asarray(x, np.float32))
    xt_blocks = []
    for c in range(NCORES):
        sel = core_pos == c
        Xp = np.zeros((NT * 128, CIN), np.float32)
        Xp[xrow[sel]] = x[sel]
        # lhsT block layout: [mt, p, k*128 + m] = Xp[mt*128+m, k*128+p]
        xt = Xp.reshape(NT, 128, KT, 128).transpose(0, 3, 2, 1).reshape(NT, 128, CIN)
        xt_blocks.append(np.ascontiguousarray(xt.astype(BF16)))

    Wi_l = np.ascontiguousarray(
        np.asarray(Wi, np.float32).reshape(KT, 128, G).astype(BF16))
    Wh8 = np.ascontiguousarray(
        np.asarray(Wh, np.float32).reshape(KT, 128, G).astype(F8))
    bbc = np.ascontiguousarray(
        np.broadcast_to(np.asarray(b, np.float32)[None, :], (128, G)).astype(BF16))
    ident = np.ascontiguousarray(np.eye(128, dtype=np.float32).astype(BF16))

    ht0_blocks = [None] * NCORES
    c0_blocks = [None] * NCORES
    if not zero_init:
        h1 = np.asarray(h1, np.float32); h2 = np.asarray(h2, np.float32)
        c1 = np.asarray(c1, np.float32); c2 = np.asarray(c2, np.float32)
        hin = np.where(sch["chain_player"][:, None] > 0, h1[sch["chain_b"]],
                       h2[sch["chain_b"]])
        cin_ = np.where(sch["chain_player"][:, None] > 0, c1[sch["chain_b"]],
                        c2[sch["chain_b"]])
        live = sch["chain_seg"] == 0
        hin = np.where(live[:, None], hin, 0.0)
        cin_ = np.where(live[:, None], cin_, 0.0)
        for c in range(NCORES):
            selc = sch["core_of_chain"] == c
            rows = sch["core_rank"][selc]
            Hp = np.zeros((M0 * 128, H), np.float32)
            Cp = np.zeros((M0 * 128, H), np.float32)
            Hp[rows] = hin[selc]
            Cp[rows] = cin_[selc]
            # transposed fp8 layout for lhsT: [p, k, col] = Hp[col, k*128+p]
            ht0 = Hp.reshape(M0 * 128, KT, 128).transpose(2, 1, 0)
            ht0_blocks[c] = np.ascontiguousarray(ht0.astype(F8))
            c0_blocks[c] = np.ascontiguousarray(Cp)

    return dict(
        sch=sch, zero_init=zero_init, Zt=Zt, NT=NT,
        xt_blocks=xt_blocks, Wi_l=Wi_l, Wh8=Wh8, bbc=bbc, ident=ident,
        ht0_blocks=ht0_blocks, c0_blocks=c0_blocks,
        core_pos=core_pos, padded_row=padded_row, T=T, B=B,
    )


# ---------------------------------------------------------------------------
# Device program
# ---------------------------------------------------------------------------

def _build_program(D, U, M, V, P, zero_init):
    nc = bacc.Bacc("TRN2", target_bir_lowering=False, debug=False)

    M_ = [int(m) for m in M]
    P_ = [int(p) for p in P]
    dz = 1 if zero_init else 0
    M0 = M_[0]
    Zt = _zx_tiles(D, M, V, zero_init)
    NT = Zt + (M0 if zero_init else 0)
    Ptot = P_[D]
    Vz = [int(V[d] - V[dz]) for d in range(D + 1)]
    have_waves = D > dz

    xt_d = nc.dram_tensor("xt", [NT, 128, CIN], DT.bfloat16, kind="ExternalInput")
    wi_d = nc.dram_tensor("wi", [KT, 128, G], DT.bfloat16, kind="ExternalInput")
    bbc_d = nc.dram_tensor("bbc", [128, G], DT.bfloat16, kind="ExternalInput")
    id_d = nc.dram_tensor("ident", [128, 128], DT.bfloat16, kind="ExternalInput")
    y_d = nc.dram_tensor("y", [Ptot, H], DT.bfloat16, kind="ExternalOutput")
    if have_waves:
        wh_d = nc.dram_tensor("wh8", [KT, 128, G], DT.float8e4,
                              kind="ExternalInput")
        zx_d = nc.dram_tensor("zx", [Zt * 128, 2, 2048], DT.bfloat16,
                              kind="Internal")
        zx_ap = zx_d.ap()
    # c-state DRAM regions for waves whose consumer wave has >1 m-tile
    cs_off = {}
    acc = 0
    for d in range(D - 1):
        if M_[d + 1] > 1:
            cs_off[d] = acc
            acc += M_[d + 1] * 128
    if acc:
        cs_d = nc.dram_tensor("cstate", [acc, H], DT.float32, kind="Internal")
        cs_ap = cs_d.ap()
    if not zero_init:
        ht0_d = nc.dram_tensor("ht0", [128, KT, M0 * 128], DT.float8e4,
                               kind="ExternalInput")
        c0_d = nc.dram_tensor("c0", [M0 * 128, H], DT.float32,
                              kind="ExternalInput")

    xt_ap = xt_d.ap()
    y_ap = y_d.ap()

    with tile.TileContext(nc) as tc:
        from contextlib import ExitStack
        with ExitStack() as es:
            const = es.enter_context(tc.tile_pool(name="const", bufs=1))
            wp = es.enter_context(tc.tile_pool(name="weights", bufs=1))
            xtP = es.enter_context(tc.tile_pool(name="xtP", bufs=3))
            zbP = es.enter_context(tc.tile_pool(name="zbP", bufs=2))
            zxP = es.enter_context(tc.tile_pool(name="zxP", bufs=4))
            gP = es.enter_context(tc.tile_pool(name="gP", bufs=1))
            fP = es.enter_context(tc.tile_pool(name="fP", bufs=2))
            sP = es.enter_context(tc.tile_pool(name="sP", bufs=2))
            hTP = es.enter_context(tc.tile_pool(name="hTP", bufs=1))
            cP = es.enter_context(tc.tile_pool(name="cP", bufs=2))
            psum = es.enter_context(tc.tile_pool(name="psum", bufs=2,
                                                 space="PSUM"))

            # ---- weights / constants (emission order = DMA priority) ----
            wi_sb = wp.tile([128, KT, G], DT.bfloat16, tag="wi")
            # k=0 on the gpsimd queue so the first matmul unblocks early;
            # the rest on the sync queue to load in parallel with xt tiles.
            nc.gpsimd.dma_start(out=wi_sb[:, 0:1, :], in_=wi_d.ap()[0])
            for k in range(1, KT):
                nc.sync.dma_start(out=wi_sb[:, k:k + 1, :], in_=wi_d.ap()[k])
            if have_waves:
                wh8_sb = wp.tile([128, KT, G], DT.float8e4, tag="wh8")
                for k in range(KT):
                    nc.sync.dma_start(out=wh8_sb[:, k:k + 1, :],
                                      in_=wh_d.ap()[k])
            ident_sb = const.tile([128, 128], DT.bfloat16, tag="ident")
            bbc_sb = const.tile([128, G], DT.bfloat16, tag="bbc")

            hT8 = {}
            for d in range(1, D):
                hT8[d] = hTP.tile([128, KT, M_[d] * 128], DT.float8e4,
                                  tag=f"hT{d}", name=f"hT{d}")
            if not zero_init:
                ht0_sb = wp.tile([128, KT, M0 * 128], DT.float8e4, tag="ht0")
                nc.gpsimd.dma_start(out=ht0_sb[:], in_=ht0_d.ap()[:])

            # gate tiles
            gi = gP.tile([128, H], DT.bfloat16, tag="gi", name="gi")
            gf = gP.tile([128, H], DT.bfloat16, tag="gf", name="gf")
            gg = gP.tile([128, H], DT.bfloat16, tag="gg", name="gg")
            go = gP.tile([128, H], DT.bfloat16, tag="go", name="go")

            keep_state = {}  # wave d -> SBUF fp32 c tile (single-tile consumer)

            def mm_phaseA(xt_sb, half, skip_f):
                pt = {}
                for g in range(4):
                    if skip_f and g == 1:
                        continue
                    pt[g] = psum.tile([128, 512], DT.float32, tag=f"ph{g}",
                                      name=f"ph{g}")
                for k in range(KT):
                    for g in pt:
                        col0 = g * H + half * 512
                        nc.tensor.matmul(
                            pt[g][:], lhsT=xt_sb[:, k:k + 1, :],
                            rhs=wi_sb[:, k:k + 1, col0:col0 + 512],
                            start=(k == 0), stop=(k == KT - 1),
                            skip_group_check=True)
                return pt

            def drain(pt, half):
                """PSUM -> SBUF bf16 with bias add (DVE; GPSIMD can't read PSUM)."""
                zbs = {}
                for g, p in pt.items():
                    col0 = g * H + half * 512
                    zb = zbP.tile([128, 512], DT.bfloat16, tag=f"zb{g}",
                                  name=f"zb{g}")
                    nc.vector.tensor_add(zb[:], p[:], bbc_sb[:, col0:col0 + 512])
                    zbs[g] = zb
                return zbs

            def zx_job(t):
                xt_sb = xtP.tile([128, KT, 128], DT.bfloat16, tag="xt",
                                 name="xt")
                nc.gpsimd.dma_start(out=xt_sb[:], in_=xt_ap[t])
                if t == 0:
                    nc.gpsimd.dma_start(out=bbc_sb[:], in_=bbc_d.ap()[:])
                    nc.gpsimd.dma_start(out=ident_sb[:], in_=id_d.ap()[:])
                for half in range(2):
                    pt = mm_phaseA(xt_sb, half, skip_f=False)
                    zbs = drain(pt, half)
                    for g in range(4):
                        nc.sync.dma_start(
                            out=zx_ap[t * 128:(t + 1) * 128, half:half + 1,
                                      g * 512:(g + 1) * 512],
                            in_=zbs[g][:])

            def w0_job(j):
                """Fused depth-0 tile (zero-init: c == 0, f gate skipped)."""
                feeds = have_waves and j < M_[1]
                t = Zt + j
                xt_sb = xtP.tile([128, KT, 128], DT.bfloat16, tag="xt",
                                 name="xt")
                nc.gpsimd.dma_start(out=xt_sb[:], in_=xt_ap[t])
                if Zt == 0 and j == 0:
                    nc.gpsimd.dma_start(out=bbc_sb[:], in_=bbc_d.ap()[:])
                    nc.gpsimd.dma_start(out=ident_sb[:], in_=id_d.ap()[:])

                if feeds and M_[1] == 1:
                    ncv = sP.tile([128, H], DT.float32, tag="keepT",
                                  name="ncv_keep", bufs=2)
                    keep_state[0] = ncv
                else:
                    ncv = sP.tile([128, H], DT.float32, tag="ncv", name="ncv")
                nh = sP.tile([128, H], DT.bfloat16, tag="nh", name="nh")
                if feeds:
                    hTb = sP.tile([128, KT, 128], DT.bfloat16, tag="hTb",
                                  name="hTb")
                for half in range(2):
                    blk = slice(half * 512, half * 512 + 512)
                    pt = mm_phaseA(xt_sb, half, skip_f=True)
                    zbs = drain(pt, half)
                    nc.scalar.activation(gi[:, blk], zbs[0][:], AF.Sigmoid)
                    nc.scalar.activation(gg[:, blk], zbs[2][:], AF.Tanh)
                    nc.gpsimd.tensor_mul(ncv[:, blk], gi[:, blk], gg[:, blk])
                    nc.scalar.activation(go[:, blk], zbs[3][:], AF.Sigmoid)
                    tancb = fP.tile([128, 512], DT.float32, tag="tanc",
                                    name="tanc")
                    nc.scalar.activation(tancb[:], ncv[:, blk], AF.Tanh)
                    nc.vector.tensor_mul(nh[:, blk], go[:, blk], tancb[:])
                    if feeds:
                        nc.sync.dma_start_transpose(
                            out=hTb[:, half * 4:(half + 1) * 4, :],
                            in_=nh[:, blk])
                        nc.vector.tensor_copy(
                            hT8[1][:, half * 4:(half + 1) * 4,
                                   j * 128:(j + 1) * 128],
                            hTb[:, half * 4:(half + 1) * 4, :])
                nc.gpsimd.dma_start(out=y_ap[j * 128:(j + 1) * 128, :],
                                    in_=nh[:])
                if feeds and M_[1] > 1:
                    nc.gpsimd.dma_start(
                        out=cs_ap[cs_off[0] + j * 128:cs_off[0] + (j + 1) * 128, :],
                        in_=ncv[:])

            def prefetch_wv(d, j):
                """Issue the DMA reads a wave tile needs (called 1 job early)."""
                zxh = []
                for half in range(2):
                    zt = zxP.tile([128, 2048], DT.bfloat16, tag="zxh",
                                  name="zxh")
                    nc.gpsimd.dma_start(
                        out=zt[:],
                        in_=zx_ap[Vz[d] + j * 128:Vz[d] + (j + 1) * 128,
                                  half:half + 1, :])
                    zxh.append(zt)
                c_src = None
                if d == 0 and not zero_init:
                    c_src = cP.tile([128, H], DT.float32, tag="c", name="c_sb")
                    nc.gpsimd.dma_start(
                        out=c_src[:], in_=c0_d.ap()[j * 128:(j + 1) * 128, :])
                elif M_[d] > 1:
                    c_src = cP.tile([128, H], DT.float32, tag="c", name="c_sb")
                    nc.gpsimd.dma_start(
                        out=c_src[:],
                        in_=cs_ap[cs_off[d - 1] + j * 128:
                                  cs_off[d - 1] + (j + 1) * 128, :])
                return zxh, c_src

            def wv_job(d, j, zxh, c_src):
                feeds = (d + 1 < D) and (j < M_[d + 1])
                if c_src is None:
                    c_src = keep_state[d - 1]
                # lhsT source
                hsrc = ht0_sb if (d == 0 and not zero_init) else hT8[d]

                if feeds and M_[d + 1] == 1:
                    ncv = sP.tile([128, H], DT.float32, tag="keepT",
                                  name="ncv_keep", bufs=2)
                    keep_state[d] = ncv
                else:
                    ncv = sP.tile([128, H], DT.float32, tag="ncv", name="ncv")
                nh = sP.tile([128, H], DT.bfloat16, tag="nh", name="nh")
                if feeds:
                    hTb = sP.tile([128, KT, 128], DT.bfloat16, tag="hTb",
                                  name="hTb")
                for half in range(2):
                    blk = slice(half * 512, half * 512 + 512)
                    pt = []
                    for g in range(4):
                        pt.append(psum.tile([128, 512], DT.float32,
                                            tag=f"ph{g}", name=f"ph{g}"))
                    for kp in range(KT // 2):
                        for g in range(4):
                            col0 = g * H + half * 512
                            nc.tensor.matmul(
                                pt[g][:],
                                lhsT=hsrc[:, 2 * kp:2 * kp + 2,
                                          j * 128:(j + 1) * 128],
                                rhs=wh8_sb[:, 2 * kp:2 * kp + 2,
                                           col0:col0 + 512],
                                start=(kp == 0), stop=False,
                                perf_mode=DR, skip_group_check=True)
                    for g in range(4):
                        nc.tensor.matmul(
                            pt[g][:], lhsT=ident_sb[:],
                            rhs=zxh[half][:, g * 512:(g + 1) * 512],
                            start=False, stop=True, skip_group_check=True)
                    # gates straight from PSUM (zx inject carries the bias)
                    nc.scalar.activation(gi[:, blk], pt[0][:], AF.Sigmoid)
                    nc.scalar.activation(gg[:, blk], pt[2][:], AF.Tanh)
                    m1b = fP.tile([128, 512], DT.float32, tag="m1", name="m1")
                    nc.vector.tensor_mul(m1b[:], gi[:, blk], gg[:, blk])
                    nc.scalar.activation(gf[:, blk], pt[1][:], AF.Sigmoid)
                    t1b = fP.tile([128, 512], DT.float32, tag="t1", name="t1")
                    nc.gpsimd.tensor_mul(t1b[:], gf[:, blk], c_src[:, blk])
                    nc.vector.tensor_add(ncv[:, blk], t1b[:], m1b[:])
                    nc.scalar.activation(go[:, blk], pt[3][:], AF.Sigmoid)
                    tancb = fP.tile([128, 512], DT.float32, tag="tanc",
                                    name="tanc")
                    nc.scalar.activation(tancb[:], ncv[:, blk], AF.Tanh)
                    nc.vector.tensor_mul(nh[:, blk], go[:, blk], tancb[:])
                    if feeds:
                        nc.sync.dma_start_transpose(
                            out=hTb[:, half * 4:(half + 1) * 4, :],
                            in_=nh[:, blk])
                        nc.vector.tensor_copy(
                            hT8[d + 1][:, half * 4:(half + 1) * 4,
                                       j * 128:(j + 1) * 128],
                            hTb[:, half * 4:(half + 1) * 4, :])
                nc.gpsimd.dma_start(
                    out=y_ap[P_[d] + j * 128:P_[d] + (j + 1) * 128, :],
                    in_=nh[:])
                if feeds and M_[d + 1] > 1:
                    nc.gpsimd.dma_start(
                        out=cs_ap[cs_off[d] + j * 128:
                                  cs_off[d] + (j + 1) * 128, :],
                        in_=ncv[:])

            # ---- job order ----
            jobs = []
            if zero_init:
                jobs += [("zx", t) for t in range(Zt)]
                head = min(M_[1], M0) if have_waves else 0
                jobs += [("w0", j) for j in range(head)]
                from collections import deque
                fill = deque(range(head, M0))
                for d in range(1, D):
                    for j in range(M_[d]):
                        jobs.append(("wv", d, j))
                    if d + 1 < D and M_[d] == 1 and fill:
                        jobs.append(("w0", fill.popleft()))
                while fill:
                    jobs.append(("w0", fill.popleft()))
            else:
                jobs += [("zx", t) for t in range(Zt)]
                for d in range(D):
                    for j in range(M_[d]):
                        jobs.append(("wv", d, j))

            # Dispatch with 1-job DMA prefetch for wave tiles so their zx/c
            # reads are issued (and in flight) a full job ahead of the PE.
            prefetched = {}
            for idx, job in enumerate(jobs):
                nxt = idx + 1
                if nxt < len(jobs) and jobs[nxt][0] == "wv" \
                        and nxt not in prefetched:
                    prefetched[nxt] = prefetch_wv(jobs[nxt][1], jobs[nxt][2])
                if job[0] == "zx":
                    zx_job(job[1])
                elif job[0] == "w0":
                    w0_job(job[1])
                else:
                    if idx not in prefetched:
                        prefetched[idx] = prefetch_wv(job[1], job[2])
                    zxh, c_src = prefetched.pop(idx)
                    wv_job(job[1], job[2], zxh, c_src)

    nc.compile()
    return nc


# ---------------------------------------------------------------------------
# Entry point
# ---------------------------------------------------------------------------

_PROGRAM_CACHE = {}


def _run(inputs, trace=False):
    prep = _prep_inputs(**inputs)
    sch = prep["sch"]
    D, U, M, V, P = sch["D"], sch["U"], sch["M"], sch["V"], sch["P"]

    in_maps = []
    for c in range(NCORES):
        m = {
            "xt": prep["xt_blocks"][c],
            "wi": prep["Wi_l"],
            "bbc": prep["bbc"],
            "ident": prep["ident"],
        }
        if D > (1 if prep["zero_init"] else 0):
            m["wh8"] = prep["Wh8"]
        if not prep["zero_init"]:
            m["ht0"] = prep["ht0_blocks"][c]
            m["c0"] = prep["c0_blocks"][c]
        in_maps.append(m)

    # Retry ladder: rare transient device errors have been observed on the
    # shared terminal; retry a couple of times.
    import time as _time
    res = None
    last_err = None
    for attempt in range(3):
        key = (D, tuple(int(v) for v in M), tuple(int(v) for v in U),
               prep["zero_init"])
        try:
            if key not in _PROGRAM_CACHE:
                _PROGRAM_CACHE[key] = _build_program(
                    D, U, M, V, P, prep["zero_init"])
            nc = _PROGRAM_CACHE[key]
            res = run_bass_kernel_spmd(nc, in_maps,
                                       core_ids=list(range(NCORES)),
                                       trace=trace)
            break
        except Exception as e:  # noqa: BLE001 - retry on device hiccups
            last_err = e
            sys.stderr.write(f"kernel attempt {attempt} failed: {e!r}\n")
            trace = False  # profiling hook may be wedged; drop it on retry
            _time.sleep(2.0)
    if res is None:
        raise last_err

    T, B = prep["T"], prep["B"]
    y_full = np.empty((T * B, H), np.float32)
    core_pos = prep["core_pos"]
    padded_row = prep["padded_row"]
    for c in range(NCORES):
        sel = core_pos == c
        y_full[sel] = res.results[c]["y"][padded_row[sel]].astype(np.float32)
    return y_full, res


def kernel(**inputs) -> np.ndarray:
    y, _ = _run(inputs, trace=False)
    return y
